# revision 5
# baseline (speedup 1.0000x reference)
"""Causal single-head attention on 8 TRN2 NeuronCores, data-parallel over batch.

Per core (one batch element): x [T=2048, C=1024], weights [C, H=128].
  q = x@Wq + bq ; k = x@Wk + bk ; v = x@Wv + bv
  out = softmax(mask(q k^T / sqrt(H))) @ v

Layout strategy (no on-device transposes anywhere):
  - host passes x^T [C, T]; projections contract C on partitions:
      qT, kT [H, T] (stationary = W[c,h]), v [T, H] (stationary = xT[c,t128])
  - scores computed transposed, S'[s, t] = k q^T, via stationary kT[:, s128]
  - softmax sums via a ones-column appended to v: one PV matmul per t-chunk
    yields both sum_s P'[s,t] v[s,h] and sum_s P'[s,t]
  - causal masking: multiplicative 0/1 bf16 masks on the 4 diagonal
    block shapes, generated on-chip with affine_select
  - matmul inputs bf16 (fp32 PSUM accumulation), everything else fp32
"""

import numpy as np

import concourse.bass as bass
import concourse.mybir as mybir
import concourse.tile as tile
from concourse.bass_utils import run_bass_kernel_spmd

F32 = mybir.dt.float32
BF16 = mybir.dt.bfloat16
AF = mybir.ActivationFunctionType

B, T, C, H = 8, 2048, 1024, 128
P = 128
CT = C // P        # 8 contraction tiles
TBLK = 512         # t-block / projection chunk width
NBLK = T // TBLK   # 4
NST = T // P       # 16 s-tiles
SCALE = 1.0 / float(np.sqrt(H))

N_CORES = 8


def _split_multiwaits(nc, max_waits=1):
    """walrus in this image rejects >1 sem wait on one instruction; hoist
    extras onto single-wait NOPs placed just before on the same engine."""
    n_new = 0
    for fn in nc.m.functions:
        for bb in fn.blocks:
            new_insts = []
            for ins in bb.instructions:
                si = ins.sync_info
                if si is not None and si.on_wait and len(si.on_wait) > max_waits:
                    waits = list(si.on_wait)
                    for w in waits[:-max_waits]:
                        n_new += 1
                        new_insts.append(
                            mybir.InstNoOp(
                                name=f"I-waitsplit-{n_new}",
                                engine=ins.engine,
                                ins=[],
                                outs=[],
                                sync_info=mybir.SyncInfo(on_wait=[w], on_update=[]),
                            )
                        )
                    ins.sync_info = mybir.SyncInfo(
                        on_wait=waits[-max_waits:],
                        on_update=list(si.on_update or []),
                    )
                new_insts.append(ins)
            bb.instructions = new_insts
    return n_new


def _build(split=True):
    nc = bass.Bass()
    xT = nc.declare_dram_parameter("xT", [C, T], F32, isOutput=False)
    wqkv = nc.declare_dram_parameter("wqkv", [C, 3 * H], F32, isOutput=False)
    bqk = nc.declare_dram_parameter("bqk", [H, 2], F32, isOutput=False)
    bv = nc.declare_dram_parameter("bv", [H], F32, isOutput=False)
    out = nc.declare_dram_parameter("out", [T, H], F32, isOutput=True)

    with (
        tile.TileContext(nc) as tc,
        tc.tile_pool(name="singles", bufs=1) as singles,
        tc.tile_pool(name="wst", bufs=1) as wst,
        tc.tile_pool(name="xst", bufs=2) as xst,
        tc.tile_pool(name="xbfp", bufs=2) as xbfp,
        tc.tile_pool(name="psbp", bufs=3) as psbp,
        tc.tile_pool(name="osbp", bufs=4) as osbp,
        tc.tile_pool(name="rsbp", bufs=4) as rsbp,
        tc.tile_pool(name="ps_qk", bufs=1, space="PSUM") as ps_qk,
        tc.tile_pool(name="ps_v", bufs=1, space="PSUM") as ps_v,
        tc.tile_pool(name="ps_s", bufs=2, space="PSUM") as ps_s,
        tc.tile_pool(name="ps_o", bufs=1, space="PSUM") as ps_o,
    ):
        # ---- constants: weights, biases, masks ----
        w_f32 = wst.tile([P, CT, 3 * H], F32)
        nc.sync.dma_start(w_f32[:], wqkv.rearrange("(o p) n -> p o n", p=P))
        w_bf = singles.tile([P, CT, 3 * H], BF16)
        nc.vector.tensor_copy(w_bf[:], w_f32[:])

        bqk_sb = singles.tile([P, 2], F32)
        nc.sync.dma_start(bqk_sb[:], bqk[:, :])
        bv_rep = singles.tile([P, H], F32)
        bv_ap = bv[:]
        nc.sync.dma_start(
            bv_rep[:],
            bass.AP(tensor=bv_ap.tensor, offset=bv_ap.offset, ap=[[0, P], [1, H]]),
        )

        # masks[r][i, t'] = 1.0 if t' >= i + 128 r else 0.0
        masks = singles.tile([P, 4, TBLK], BF16)
        nc.vector.memset(masks[:], 1.0)
        for r in range(4):
            nc.gpsimd.affine_select(
                out=masks[:, r, :],
                in_=masks[:, r, :],
                compare_op=mybir.AluOpType.is_ge,
                fill=0.0,
                base=-(P * r),
                pattern=[[1, TBLK]],
                channel_multiplier=-1,
            )

        qT_sb = singles.tile([P, T], BF16)   # [h, t]
        kT_sb = singles.tile([P, T], BF16)   # [h, t]
        v_sb = singles.tile([P, NST, 132], BF16)  # [s128, s-tile, h | ones]
        nc.vector.memset(v_sb[:], 1.0)

        for j in range(NBLK):
            t0 = j * TBLK

            # ---- projections for t-chunk j ----
            x_f32 = xst.tile([P, CT, TBLK], F32, tag="x_f32")
            nc.sync.dma_start(
                x_f32[:], xT[:, t0 : t0 + TBLK].rearrange("(o p) t -> p o t", p=P)
            )
            x_bf = xbfp.tile([P, CT, TBLK], BF16, tag="x_bf")
            for o in range(CT):
                nc.vector.tensor_copy(x_bf[:, o, :], x_f32[:, o, :])

            pq = ps_qk.tile([P, TBLK], F32, tag="pqk")
            for o in range(CT):
                nc.tensor.matmul(
                    pq[:], w_bf[:, o, 0:H], x_bf[:, o, :],
                    start=(o == 0), stop=(o == CT - 1),
                )
            nc.scalar.activation(
                qT_sb[:, t0 : t0 + TBLK], pq[:], AF.Identity, bias=bqk_sb[:, 0:1]
            )

            pv = ps_v.tile([P, 4, H], F32, tag="pv")
            for m4 in range(4):
                for o in range(CT):
                    nc.tensor.matmul(
                        pv[:, m4, :],
                        x_bf[:, o, m4 * P : (m4 + 1) * P],
                        w_bf[:, o, 2 * H : 3 * H],
                        start=(o == 0), stop=(o == CT - 1),
                    )
            for m4 in range(4):
                nc.scalar.activation(v_sb[:, 4 * j + m4, 0:H], pv[:, m4, :], AF.Copy)

            pk = ps_qk.tile([P, TBLK], F32, tag="pqk")
            for o in range(CT):
                nc.tensor.matmul(
                    pk[:], w_bf[:, o, H : 2 * H], x_bf[:, o, :],
                    start=(o == 0), stop=(o == CT - 1),
                )
            nc.scalar.activation(
                kT_sb[:, t0 : t0 + TBLK], pk[:], AF.Identity, bias=bqk_sb[:, 1:2]
            )

            # ---- attention for t-block j ----
            po_tiles = [
                ps_o.tile([P, 132], F32, tag=f"po{c}", name=f"po{c}")
                for c in range(4)
            ]
            n_s = 4 * (j + 1)

            def pv_mms(m, p_sb):
                for c in range(4):
                    nc.tensor.matmul(
                        po_tiles[c][:, 0 : H + 1],
                        p_sb[:, c * P : (c + 1) * P],
                        v_sb[:, m, 0 : H + 1],
                        start=(m == 0), stop=(m == n_s - 1),
                    )

            prev = None
            for m in range(n_s):
                ps = ps_s.tile([P, TBLK], F32, tag="ps")
                nc.tensor.matmul(
                    ps[:], kT_sb[:, m * P : (m + 1) * P], qT_sb[:, t0 : t0 + TBLK],
                    start=True, stop=True,
                )
                p_sb = psbp.tile([P, TBLK], BF16, tag="p_sb")
                nc.scalar.activation(p_sb[:], ps[:], AF.Exp, scale=SCALE)
                r = m - 4 * j
                if r >= 0:
                    nc.vector.tensor_mul(p_sb[:], p_sb[:], masks[:, r, :])
                if prev is not None:
                    pv_mms(*prev)
                prev = (m, p_sb)
            pv_mms(*prev)

            # ---- epilogue: normalize + bias + store ----
            for c in range(4):
                po = po_tiles[c]
                rec = rsbp.tile([P, 1], F32, tag="rec")
                nc.vector.reciprocal(rec[:], po[:, H : H + 1])
                o_sb = osbp.tile([P, H], F32, tag="o_sb")
                nc.vector.tensor_scalar_mul(o_sb[:], po[:, 0:H], rec[:])
                nc.vector.tensor_add(o_sb[:], o_sb[:], bv_rep[:])
                nc.sync.dma_start(out[t0 + c * P : t0 + (c + 1) * P, :], o_sb[:])

    if split:
        _split_multiwaits(nc)
    return nc


_NC = None


def _get_nc():
    global _NC
    if _NC is None:
        _NC = _build()
    return _NC


def _prepare_in_maps(batch_x, Wq, bq, Wk, bk, Wv, bv):
    wqkv = np.ascontiguousarray(
        np.concatenate(
            [np.asarray(Wq), np.asarray(Wk), np.asarray(Wv)], axis=1
        ).astype(np.float32)
    )
    bqk = np.ascontiguousarray(
        np.stack([np.asarray(bq), np.asarray(bk)], axis=1).astype(np.float32)
    )
    bvv = np.ascontiguousarray(np.asarray(bv).astype(np.float32))
    bx = np.asarray(batch_x)
    return [
        {
            "xT": np.ascontiguousarray(bx[i].T.astype(np.float32)),
            "wqkv": wqkv,
            "bqk": bqk,
            "bv": bvv,
        }
        for i in range(N_CORES)
    ]


def kernel(batch_x, Wq, bq, Wk, bk, Wv, bv):
    nc = _get_nc()
    in_maps = _prepare_in_maps(batch_x, Wq, bq, Wk, bk, Wv, bv)
    res = run_bass_kernel_spmd(nc, in_maps, core_ids=list(range(N_CORES)))
    return np.stack([res.results[i]["out"] for i in range(N_CORES)], axis=0)
